# revision 1
# baseline (speedup 1.0000x reference)
"""DND-LSTM cell (retrieval kNN + LSTM gates) on 8 Trainium2 NeuronCores.

Strategy (sharding_hint): shard keys/vals along dict_len (L=100000) across the
8 cores, 12500 each (padded to 12544 with dummy unit keys, excluded from the
softmax sums via ragged matmul slices). Each core streams its keysT/vals shard
from HBM once (memory-bound regime) and computes, flash-softmax style:

  num_partial[b, h]  = sum_l exp(cos(q_b, k_l) - 1) * vals[l, h]
  den_partial[b]     = sum_l exp(cos(q_b, k_l) - 1)

(cosine <= 1 exactly, so "-1" replaces the running row-max of a standard
streaming softmax; num/den ratios are unchanged.) The small LSTM-gate GEMMs are
sharded over the hidden dim (each core computes the 5 gate slices for its 32
hidden columns). The host gathers: sums the 8 num/den partials (the all-reduce)
and applies the final elementwise combine.

Device dataflow per 2048-key block (per core):
  DMA  keysT [128, 2, 2048] fp32 + vals [128, 16, 256] fp32   (one DMA each)
  POOL kt16 = bf16(keysT); vb[:, :, 0:256] = bf16(vals)       (idle engine;
       col 256 of vb = 1.0 so the same matmul accumulates the denominator)
  DVE  sq = kt16 * kt16 (bf16 2x mode)
  PE   normsq chunks [1, 512] = ones.T @ sq                   (bf16 matmul)
  ACT/DVE copy chunks -> SBUF [1, 2048]; POOL-DMA reshape -> [16, 128];
  PE   tiny transpose -> [128, 16] psum
  ACT  rsq = exp(-0.5 * ln(normsq))      (rsqrt; everything on ACT uses ONE
       table - natural_log_exp_and_others - so no 1.3us table reloads)
  PE   simsT[l, b] = kt16_tile.T @ qnT                        (bf16, N=256)
  ACT  ex = exp(simsT * rsq[l] - 1) -> bf16                   (fused scale+bias)
  PE   av[b, 0:258] += ex_bhalf.T @ vb_tile                   (persistent PSUM)

All heavy matmuls are bf16 (fp32 inputs rounded on device; the retrieval
branch contributes ~3e-3 of the output magnitude so bf16 rounding is far
below tolerance), the LSTM-gate matmuls stay float32r. Sigmoid/tanh are
computed as exp/ln compositions to stay on the single ACT table and avoid
custom DVE ucode. The only host arithmetic is the 8-way partial sum + final
elementwise LSTM combine (~0.002% of FLOPs).
"""

import ml_dtypes
import numpy as np

import concourse.bacc as bacc
import concourse.hw_specs as hw_specs
import concourse.masks as masks
import concourse.mybir as mybir
import concourse.tile as tile
from concourse import bass_utils

F32 = mybir.dt.float32
F32R = mybir.dt.float32r
BF16 = mybir.dt.bfloat16
AF = mybir.ActivationFunctionType

B = 256
D = 256
H = 256
NCORES = 8
HS = H // NCORES          # 32 hidden cols per core
GS = 5 * HS               # 160 gate cols per core
L = 100000
L_LOC = L // NCORES       # 12500 real keys per core
BLK = 1024                # keys per stream block
LPAD = ((L_LOC + 127) // 128) * 128  # 12544
NT_MAX = BLK // 128       # 16 l-tiles per full block

_TABLES_PATCHED = False


def _patch_act_tables():
    """Make every ACT function resolve to the one table that holds Ln, Exp,
    Square and Copy together (natural_log_exp_and_others). The default
    first-fit choice alternates between two tables, costing a 1.28us
    ACT_TABLE_LOAD per switch inside the hot loop. Table *indices* are
    preserved (ids index act_info.json), only membership is masked."""
    global _TABLES_PATCHED
    if _TABLES_PATCHED:
        return
    _TABLES_PATCHED = True
    orig = bacc.get_activation_tables

    def patched(arch):
        t = dict(orig(arch))
        keep = "natural_log_exp_and_others"
        if keep in t:
            t = {name: (funcs if name == keep else set())
                 for name, funcs in t.items()}
        return t

    bacc.get_activation_tables = patched


def _build(l_real=L_LOC, lpad=LPAD, blk=BLK):
    """Emit the per-core Bass program (identical on all 8 cores; all per-core
    variation is in the input data)."""
    _patch_act_tables()
    nt_max = blk // 128
    nblk_full = lpad // blk
    tail = lpad - nblk_full * blk
    blocks = [blk] * nblk_full + ([tail] if tail else [])

    nc = bacc.Bacc("TRN2", target_bir_lowering=False, debug=False,
                   num_devices=NCORES)

    keysT = nc.dram_tensor("keysT", [D, lpad], F32, kind="ExternalInput")
    nblk = len(blocks)
    vals2 = nc.dram_tensor("vals2", [nblk, 128, nt_max * (H + 2)], F32,
                           kind="ExternalInput")
    x_t = nc.dram_tensor("x_t", [B, D], F32, kind="ExternalInput")
    xT_aug = nc.dram_tensor("xT_aug", [D + 2, B], F32R, kind="ExternalInput")
    hT = nc.dram_tensor("hT", [H, B], F32R, kind="ExternalInput")
    WiT = nc.dram_tensor("WiT", [D + 2, GS], F32R, kind="ExternalInput")
    WhT = nc.dram_tensor("WhT", [H, GS], F32R, kind="ExternalInput")
    c_sl = nc.dram_tensor("c_sl", [B, HS], F32, kind="ExternalInput")
    onesc = nc.dram_tensor("onesc", [128, 32], BF16, kind="ExternalInput")

    nd = nc.dram_tensor("nd", [B, H + 2], F32, kind="ExternalOutput")
    org = nc.dram_tensor("org", [B, 3 * HS], F32, kind="ExternalOutput")

    with tile.TileContext(nc) as tc:
        with (
            tc.tile_pool(name="const", bufs=1) as const,
            tc.tile_pool(name="sbA", bufs=2) as sbA,
            tc.tile_pool(name="psA", bufs=1, space="PSUM") as psA,
            tc.tile_pool(name="kpool", bufs=3) as kpool,
            tc.tile_pool(name="k16pool", bufs=3) as k16pool,
            tc.tile_pool(name="vbpool", bufs=3) as vbpool,
            tc.tile_pool(name="v16pool", bufs=4) as v16pool,
            tc.tile_pool(name="sqpool", bufs=3) as sqpool,
            tc.tile_pool(name="nqps", bufs=2, space="PSUM") as nqps,
            tc.tile_pool(name="rqpool", bufs=3) as rqpool,
            tc.tile_pool(name="smps", bufs=3, space="PSUM") as smps,
            tc.tile_pool(name="expool", bufs=6) as expool,
            tc.tile_pool(name="avps", bufs=1, space="PSUM") as avps,
        ):
            # --- constants ---
            ident = const.tile([128, 128], F32)
            masks.make_identity(nc, ident[:])
            ones32 = const.tile([128, 32], BF16)
            nc.sync.dma_start(ones32[:], onesc.ap()[:])
            cm1 = const.tile([128, 1], F32)
            nc.vector.memset(cm1[:], -1.0)
            cm2 = const.tile([128, 1], F32)
            nc.vector.memset(cm2[:], -2.0)
            cm05 = const.tile([128, 1], F32)
            nc.vector.memset(cm05[:], -0.5)

            # --- phase A: qn = x / ||x||, then qnT via PE transpose ---
            qnT = [const.tile([128, B], BF16, tag=f"qnT{dc}", name=f"qnT{dc}")
                   for dc in range(2)]
            for bh in range(2):
                xt = sbA.tile([128, D], F32, tag="xt")
                nc.sync.dma_start(xt[:],
                                  x_t.ap()[bh * 128:(bh + 1) * 128, :])
                scr = sbA.tile([128, D], F32, tag="scr")
                nsq = sbA.tile([128, 1], F32, tag="nsq")
                nc.scalar.activation(scr[:], xt[:], AF.Square,
                                     accum_out=nsq[:])
                lnx = sbA.tile([128, 1], F32, tag="lnx")
                nc.scalar.activation(lnx[:], nsq[:], AF.Ln)
                rsx = sbA.tile([128, 1], F32, tag="rsx")
                nc.scalar.activation(rsx[:], lnx[:], AF.Exp, scale=cm05[:])
                qn = sbA.tile([128, D], F32, tag="qn")
                nc.vector.tensor_scalar_mul(qn[:], xt[:], rsx[:])
                for dc in range(2):
                    tp = psA.tile([128, 128], F32, tag="ps_scratch", name="tp")
                    nc.tensor.transpose(
                        tp[:], qn[:, dc * 128:(dc + 1) * 128], ident[:])
                    nc.vector.tensor_copy(
                        qnT[dc][:, bh * 128:(bh + 1) * 128], tp[:])

            # --- phase B: LSTM gate slices (this core's 32 hidden cols) ---
            xa = [sbA.tile([128, B], F32R, tag=f"xa{i}", name=f"xa{i}")
                  for i in range(2)]
            xa2 = sbA.tile([2, B], F32R, tag="xa2")
            ha = [sbA.tile([128, B], F32R, tag=f"ha{i}", name=f"ha{i}")
                  for i in range(2)]
            wi = [sbA.tile([128, GS], F32R, tag=f"wi{i}", name=f"wi{i}")
                  for i in range(2)]
            wi2 = sbA.tile([2, GS], F32R, tag="wi2")
            wh = [sbA.tile([128, GS], F32R, tag=f"wh{i}", name=f"wh{i}")
                  for i in range(2)]
            ctile = [sbA.tile([128, HS], F32, tag=f"ct{i}", name=f"ct{i}")
                     for i in range(2)]
            for i in range(2):
                nc.sync.dma_start(xa[i][:],
                                  xT_aug.ap()[i * 128:(i + 1) * 128, :])
                nc.sync.dma_start(ha[i][:],
                                  hT.ap()[i * 128:(i + 1) * 128, :])
                nc.sync.dma_start(wi[i][:],
                                  WiT.ap()[i * 128:(i + 1) * 128, :])
                nc.sync.dma_start(wh[i][:],
                                  WhT.ap()[i * 128:(i + 1) * 128, :])
                nc.sync.dma_start(
                    ctile[i][:], c_sl.ap()[i * 128:(i + 1) * 128, :])
            nc.sync.dma_start(xa2[:], xT_aug.ap()[256:258, :])
            nc.sync.dma_start(wi2[:], WiT.ap()[256:258, :])

            for bh in range(2):
                bsl = slice(bh * 128, (bh + 1) * 128)
                pre = psA.tile([128, GS], F32, tag="ps_scratch", name="pre")
                nc.tensor.matmul(pre[:], xa[0][:, bsl], wi[0][:],
                                 start=True, stop=False)
                nc.tensor.matmul(pre[:], xa[1][:, bsl], wi[1][:],
                                 start=False, stop=False)
                nc.tensor.matmul(pre[:], xa2[:, bsl], wi2[:],
                                 start=False, stop=False)
                nc.tensor.matmul(pre[:], ha[0][:, bsl], wh[0][:],
                                 start=False, stop=False)
                nc.tensor.matmul(pre[:], ha[1][:, bsl], wh[1][:],
                                 start=False, stop=True)
                gates = sbA.tile([128, GS], F32, tag="gates")
                # sigmoid(x) = exp(-ln(1 + exp(-x))): stays on the Ln/Exp ACT
                # table and avoids custom DVE ucode (reciprocal) entirely
                e1 = sbA.tile([128, 128], F32, tag="e1")
                nc.scalar.activation(e1[:], pre[:, 0:128], AF.Exp, scale=cm1[:])
                nc.vector.tensor_scalar_add(e1[:], e1[:], 1.0)
                l1 = sbA.tile([128, 128], F32, tag="l1")
                nc.scalar.activation(l1[:], e1[:], AF.Ln)
                nc.scalar.activation(gates[:, 0:128], l1[:], AF.Exp,
                                     scale=cm1[:])
                # tanh(x) = 2 * sigmoid(2x) - 1
                e2 = sbA.tile([128, HS], F32, tag="e2")
                nc.scalar.activation(e2[:], pre[:, 128:160], AF.Exp,
                                     scale=cm2[:])
                nc.vector.tensor_scalar_add(e2[:], e2[:], 1.0)
                l2 = sbA.tile([128, HS], F32, tag="l2")
                nc.scalar.activation(l2[:], e2[:], AF.Ln)
                e3 = sbA.tile([128, HS], F32, tag="e3")
                nc.scalar.activation(e3[:], l2[:], AF.Exp, scale=cm1[:])
                nc.vector.tensor_scalar(
                    gates[:, 128:160], e3[:], 2.0, -1.0,
                    op0=mybir.AluOpType.mult, op1=mybir.AluOpType.add)
                # c_part = f*c + i*c~
                fc = sbA.tile([128, HS], F32, tag="fc")
                nc.vector.tensor_mul(fc[:], gates[:, 0:HS], ctile[bh][:])
                ic = sbA.tile([128, HS], F32, tag="ic")
                nc.vector.tensor_mul(ic[:], gates[:, HS:2 * HS],
                                     gates[:, 128:160])
                cp = sbA.tile([128, HS], F32, tag="cp")
                nc.vector.tensor_add(cp[:], fc[:], ic[:])
                nc.sync.dma_start(org.ap()[bsl, 0:HS],
                                  gates[:, 2 * HS:3 * HS])      # o
                nc.sync.dma_start(org.ap()[bsl, HS:2 * HS],
                                  gates[:, 3 * HS:4 * HS])      # r
                nc.sync.dma_start(org.ap()[bsl, 2 * HS:3 * HS], cp[:])

            # --- phase C: stream the kNN retrieval ---
            # Software-pipelined: block i+1's loads, casts and rsqrt-norm
            # chain (a ~9us serial latency: DMA -> cast -> sq -> PE ones-mm
            # -> copy -> reshape-DMA -> PE transpose -> ACT ln/exp) are
            # emitted BEFORE block i's tile loop, so the chain hides under
            # the previous block's matmul work and PE never idles long
            # enough for the HAM clock-gate to re-throttle.
            av = [avps.tile([128, H + 2], F32, tag=f"av{bh}", name=f"av{bh}")
                  for bh in range(2)]
            total_tiles = lpad // 128

            def emit_front(bi):
                """DMA + casts + rsqrt-norm chain for block bi."""
                bs = blocks[bi]
                off = bi * blk
                nt = bs // 128
                kt = kpool.tile([128, 2, bs], F32, tag="kt", name="kt")
                nc.sync.dma_start(
                    kt[:],
                    keysT.ap()[:, off:off + bs].rearrange(
                        "(c p) l -> p c l", p=128))
                kt16 = k16pool.tile([128, 2, bs], BF16, tag="kt16",
                                    name="kt16")
                nc.vector.tensor_copy(kt16[:], kt[:])
                vb = vbpool.tile([128, nt_max * (H + 2)], F32, tag="vb",
                                 name="vb")
                nc.sync.dma_start(vb[:, 0:nt * (H + 2)],
                                  vals2.ap()[bi, :, 0:nt * (H + 2)])
                vb16 = v16pool.tile([128, nt_max * (H + 2)], BF16, tag="vb16",
                                    name="vb16")
                nc.vector.tensor_copy(vb16[:, 0:nt * (H + 2)],
                                      vb[:, 0:nt * (H + 2)])
                sq = sqpool.tile([128, 2, bs], BF16, tag="sq", name="sq")
                nc.vector.tensor_mul(sq[:], kt16[:], kt16[:])
                # normsq[l] -> rsq[p, t] (= 1/||k_l||, l = 128*t + p):
                # chunk sums [1, 512] -> SBUF [1, bs] -> reshape-DMA
                # [nt, 128] -> tiny PE transpose -> [128, nt] psum -> rsqrt
                chunks = [(j0, min(512, bs - j0))
                          for j0 in range(0, bs, 512)]
                nqs = rqpool.tile([1, blk], F32, tag="nqs", name="nqs")
                for j, (j0, cs) in enumerate(chunks):
                    nq = nqps.tile([1, 512], F32, tag="nq", name="nq")
                    nc.tensor.matmul(nq[:, 0:cs], ones32[:, 0:1],
                                     sq[:, 0, j0:j0 + cs],
                                     start=True, stop=False)
                    nc.tensor.matmul(nq[:, 0:cs], ones32[:, 0:1],
                                     sq[:, 1, j0:j0 + cs],
                                     start=False, stop=True)
                    if j % 2 == 0:
                        nc.vector.tensor_copy(nqs[:, j0:j0 + cs],
                                              nq[:, 0:cs])
                    else:
                        nc.scalar.copy(nqs[:, j0:j0 + cs], nq[:, 0:cs])
                t4 = rqpool.tile([16, 128], F32, tag="t4", name="t4")
                nc.gpsimd.dma_start(
                    t4[0:nt, :],
                    nqs[0:1, 0:bs].rearrange("o (r p) -> o r p", p=128))
                tpn = psA.tile([128, 16], F32, tag="ps_scratch", name="tpn")
                nc.tensor.transpose(tpn[:, 0:nt], t4[0:nt, :],
                                    ident[0:nt, 0:nt])
                rsq = rqpool.tile([128, nt_max], F32, tag="rsq", name="rsq")
                rln = rqpool.tile([128, nt_max], F32, tag="rln", name="rln")
                nc.scalar.activation(rln[:, 0:nt], tpn[:, 0:nt], AF.Ln)
                nc.scalar.activation(rsq[:, 0:nt], rln[:, 0:nt], AF.Exp,
                                     scale=cm05[:])
                return kt16, vb16, rsq

            LOOKAHEAD = 2
            fronts = [emit_front(i)
                      for i in range(min(LOOKAHEAD, len(blocks)))]
            tile_idx = 0
            for bi, bs in enumerate(blocks):
                kt16, vb16, rsq = fronts[bi]
                if bi + LOOKAHEAD < len(blocks):
                    fronts.append(emit_front(bi + LOOKAHEAD))
                off = bi * blk
                nt = bs // 128
                for t in range(nt):
                    l0 = off + t * 128
                    real = min(128, max(0, l_real - l0))
                    sm = smps.tile([128, B], F32, tag="sm")
                    nc.tensor.matmul(sm[:], kt16[:, 0, t * 128:(t + 1) * 128],
                                     qnT[0][:], start=True, stop=False)
                    nc.tensor.matmul(sm[:], kt16[:, 1, t * 128:(t + 1) * 128],
                                     qnT[1][:], start=False, stop=True)
                    ex = expool.tile([128, B], BF16, tag="ex")
                    nc.scalar.activation(ex[:], sm[:], AF.Exp,
                                         bias=cm1[:], scale=rsq[:, t:t + 1])
                    if real <= 0:
                        tile_idx += 1
                        continue
                    first = tile_idx == 0
                    last = tile_idx == total_tiles - 1
                    for bh in range(2):
                        nc.tensor.matmul(
                            av[bh][:],
                            ex[0:real, bh * 128:(bh + 1) * 128],
                            vb16[0:real, t * (H + 2):(t + 1) * (H + 2)],
                            start=first, stop=last)
                    tile_idx += 1

            for bh in range(2):
                avs = sbA.tile([128, H + 2], F32, tag="avs")
                nc.vector.tensor_copy(avs[:], av[bh][:])
                nc.sync.dma_start(nd.ap()[bh * 128:(bh + 1) * 128, :],
                                  avs[:])

    nc.compile()
    return nc


_NC_CACHE = {}


def _get_nc():
    if "nc" not in _NC_CACHE:
        _NC_CACHE["nc"] = _build()
    return _NC_CACHE["nc"]


def _shard_inputs(x_t, h, c, W_i2h, b_i2h, W_h2h, b_h2h, keys, vals):
    f = np.float32
    x_t = np.ascontiguousarray(np.asarray(x_t, f))
    h = np.asarray(h, f)
    c = np.asarray(c, f)
    W_i2h = np.asarray(W_i2h, f)
    b_i2h = np.asarray(b_i2h, f)
    W_h2h = np.asarray(W_h2h, f)
    b_h2h = np.asarray(b_h2h, f)
    keys = np.asarray(keys, f)
    vals = np.asarray(vals, f)

    xT_aug = np.ascontiguousarray(
        np.concatenate([x_t.T, np.ones((2, B), f)], axis=0))
    hT = np.ascontiguousarray(h.T)
    WiT_full = W_i2h.T  # [D, G]
    WhT_full = W_h2h.T  # [H, G]

    in_maps = []
    for k in range(NCORES):
        sl = slice(k * L_LOC, (k + 1) * L_LOC)
        keysT = np.zeros((D, LPAD), f)
        keysT[:, :L_LOC] = keys[sl].T
        keysT[0, L_LOC:] = 1.0  # dummy unit keys (excluded from the sums)
        vpad = np.zeros((LPAD, H + 2), f)
        vpad[:L_LOC, :H] = vals[sl]
        vpad[:L_LOC, H] = 1.0  # denominator column (excluded rows stay 0)
        nblk = (LPAD + BLK - 1) // BLK
        v2 = np.zeros((nblk, 128, NT_MAX * (H + 2)), f)
        for bi in range(nblk):
            bs = min(BLK, LPAD - bi * BLK)
            nt = bs // 128
            blkv = vpad[bi * BLK:bi * BLK + bs]          # [bs, 258]
            v2[bi, :, :nt * (H + 2)] = blkv.reshape(
                nt, 128, H + 2).transpose(1, 0, 2).reshape(128, nt * (H + 2))
        gcols = np.concatenate(
            [np.arange(j * H + k * HS, j * H + (k + 1) * HS)
             for j in range(5)])
        WiT = np.concatenate(
            [WiT_full[:, gcols], b_i2h[gcols][None, :],
             b_h2h[gcols][None, :]], axis=0)
        in_maps.append({
            "onesc": np.ones((128, 32), ml_dtypes.bfloat16),
            "keysT": np.ascontiguousarray(keysT),
            "vals2": v2,
            "x_t": x_t,
            "xT_aug": xT_aug,
            "hT": hT,
            "WiT": np.ascontiguousarray(WiT),
            "WhT": np.ascontiguousarray(WhT_full[:, gcols]),
            "c_sl": np.ascontiguousarray(c[:, k * HS:(k + 1) * HS]),
        })
    return in_maps


def kernel(x_t, h, c, W_i2h, b_i2h, W_h2h, b_h2h, keys, vals):
    nc = _get_nc()
    in_maps = _shard_inputs(x_t, h, c, W_i2h, b_i2h, W_h2h, b_h2h, keys, vals)
    res = bass_utils.run_bass_kernel_spmd(
        nc, in_maps, core_ids=list(range(NCORES)))

    num = np.zeros((B, H), np.float64)
    den = np.zeros((B,), np.float64)
    for k in range(NCORES):
        ndk = res.results[k]["nd"]
        num += ndk[:, :H]
        den += ndk[:, H]
    m = np.tanh(num / den[:, None]).astype(np.float32)

    h_t = np.empty((B, H), np.float32)
    c_t = np.empty((B, H), np.float32)
    for k in range(NCORES):
        orgk = res.results[k]["org"]
        o = orgk[:, 0:HS]
        r = orgk[:, HS:2 * HS]
        cp = orgk[:, 2 * HS:3 * HS]
        hs = slice(k * HS, (k + 1) * HS)
        ct = cp + r * m[:, hs]
        c_t[:, hs] = ct
        h_t[:, hs] = o * np.tanh(ct)
    return (h_t, c_t)



# revision 2
# speedup vs baseline: 1.8834x; 1.8834x over previous
"""DND-LSTM cell (retrieval kNN + LSTM gates) on 8 Trainium2 NeuronCores.

Strategy: shard keys/vals along dict_len (L=100000), 12500/core (zero-padded
to 12544). Keys are normalized, scaled by 16 and cast to fp8e4 on the host, so
the device streams 6.5MB/core instead of 25.6MB (memory-bound regime) and
needs no on-device norm chain at all. Queries are normalized+scaled on the
host too (bf16). Flash-softmax with the constant shift "-1" (cosine <= 1):

  ex[l, b]   = exp(sims[l, b] - 1)          sims = (16 kn_l) . (16 qn_b) / 256
  num[b, :]  += ex[l, b] * vals[l, :]        (fp8 vals, f32 PSUM accumulate)
  den[b]     += ex[l, b]                     (via an all-ones vals column)

Per-core device program, per 2048-key block (one contiguous fp8 DMA each for
keysT [128, 2, 2048] and vals [128, 16, 258]):
  PE   sm[128l, 256b] += kt8[:, c, tile].T @ qnT[c]     (fp8 x bf16, c = 0, 1)
  ACT  ex = exp(sm / 256 - 1) -> fp8, batched 4 l-tiles per ACTIVATE
       (the +352cyc per-instruction overhead amortizes: 287ns/tile vs 507)
  PE   av[b, 0:258] += ex_tile.T @ v8_tile               (persistent PSUM)

The zero-padded tail rows contribute exactly 0 (vals rows incl. den column
are 0), so no ragged matmuls are needed. LSTM gates are sharded over hidden
dim (32 cols/core), bf16 weights, sigmoid/tanh via the native Tanh spline so
the whole kernel uses ONE ACT table (exp_and_others: Exp + Tanh). The host
sums the 8 num/den partials and applies the final elementwise combine.
"""

import ml_dtypes
import numpy as np

import concourse.bacc as bacc
import concourse.mybir as mybir
import concourse.tile as tile
from concourse import bass_utils

F32 = mybir.dt.float32
BF16 = mybir.dt.bfloat16
F8 = mybir.dt.float8e4
AF = mybir.ActivationFunctionType

B = 256
D = 256
H = 256
NCORES = 8
HS = H // NCORES          # 32 hidden cols per core
GS = 5 * HS               # 160 gate cols per core
L = 100000
L_LOC = L // NCORES       # 12500 real keys per core
LPAD = ((L_LOC + 127) // 128) * 128  # 12544
BLK = 2048                # keys per stream block
NT_ALL = LPAD // 128      # 98 l-tiles
GRP = 4                   # l-tiles per batched exp
SCALE = 16.0              # host scale on kn/qn; exp scale = 1/SCALE^2
EPS = 1e-8

_TABLES_PATCHED = False


def _patch_act_tables():
    """Resolve every ACT function to exp_and_others (has Exp AND Tanh), so
    the kernel performs exactly one 2.7us ACT_TABLE_LOAD."""
    global _TABLES_PATCHED
    if _TABLES_PATCHED:
        return
    _TABLES_PATCHED = True
    orig = bacc.get_activation_tables

    def patched(arch):
        t = dict(orig(arch))
        keep = "exp_and_others"
        if keep in t:
            t = {name: (funcs if name == keep else set())
                 for name, funcs in t.items()}
        return t

    bacc.get_activation_tables = patched


def _blocks():
    out = []
    off = 0
    while off < LPAD:
        bs = min(BLK, LPAD - off)
        out.append((off, bs))
        off += bs
    return out


def _build():
    _patch_act_tables()
    nc = bacc.Bacc("TRN2", target_bir_lowering=False, debug=False,
                   num_devices=NCORES)

    k8 = nc.dram_tensor("k8", [128, 2 * LPAD], F8, kind="ExternalInput")
    v8 = nc.dram_tensor("v8", [128, NT_ALL * (H + 2)], F8,
                        kind="ExternalInput")
    qnT8 = nc.dram_tensor("qnT8", [2, 128, B], BF16, kind="ExternalInput")
    xaT = nc.dram_tensor("xaT", [D + 2, B], BF16, kind="ExternalInput")
    hT = nc.dram_tensor("hT", [H, B], BF16, kind="ExternalInput")
    WiT = nc.dram_tensor("WiT", [D + 2, GS], BF16, kind="ExternalInput")
    WhT = nc.dram_tensor("WhT", [H, GS], BF16, kind="ExternalInput")
    c_sl = nc.dram_tensor("c_sl", [B, HS], F32, kind="ExternalInput")

    nd = nc.dram_tensor("nd", [B, H + 2], F32, kind="ExternalOutput")
    org = nc.dram_tensor("org", [B, 3 * HS], F32, kind="ExternalOutput")

    with tile.TileContext(nc) as tc:
        with (
            tc.tile_pool(name="const", bufs=1) as const,
            tc.tile_pool(name="sbA", bufs=2) as sbA,
            tc.tile_pool(name="psA", bufs=1, space="PSUM") as psA,
            tc.tile_pool(name="kpool", bufs=3) as kpool,
            tc.tile_pool(name="vpool", bufs=3) as vpool,
            tc.tile_pool(name="smps", bufs=2, space="PSUM") as smps,
            tc.tile_pool(name="expool", bufs=3) as expool,
            tc.tile_pool(name="avps", bufs=1, space="PSUM") as avps,
        ):
            cm1 = const.tile([128, 1], F32)
            nc.vector.memset(cm1[:], -1.0)
            qnT = [const.tile([128, B], BF16, tag=f"qnT{c}", name=f"qnT{c}")
                   for c in range(2)]
            for c in range(2):
                nc.sync.dma_start(qnT[c][:], qnT8.ap()[c])

            # --- LSTM gate slices (this core's 32 hidden cols) ---
            xa = [sbA.tile([128, B], BF16, tag=f"xa{i}", name=f"xa{i}")
                  for i in range(2)]
            xa2 = sbA.tile([2, B], BF16, tag="xa2")
            ha = [sbA.tile([128, B], BF16, tag=f"ha{i}", name=f"ha{i}")
                  for i in range(2)]
            wi = [sbA.tile([128, GS], BF16, tag=f"wi{i}", name=f"wi{i}")
                  for i in range(2)]
            wi2 = sbA.tile([2, GS], BF16, tag="wi2")
            wh = [sbA.tile([128, GS], BF16, tag=f"wh{i}", name=f"wh{i}")
                  for i in range(2)]
            ctile = [sbA.tile([128, HS], F32, tag=f"ct{i}", name=f"ct{i}")
                     for i in range(2)]
            for i in range(2):
                nc.sync.dma_start(xa[i][:],
                                  xaT.ap()[i * 128:(i + 1) * 128, :])
                nc.sync.dma_start(ha[i][:],
                                  hT.ap()[i * 128:(i + 1) * 128, :])
                nc.sync.dma_start(wi[i][:],
                                  WiT.ap()[i * 128:(i + 1) * 128, :])
                nc.sync.dma_start(wh[i][:],
                                  WhT.ap()[i * 128:(i + 1) * 128, :])
                nc.sync.dma_start(
                    ctile[i][:], c_sl.ap()[i * 128:(i + 1) * 128, :])
            nc.sync.dma_start(xa2[:], xaT.ap()[256:258, :])
            nc.sync.dma_start(wi2[:], WiT.ap()[256:258, :])

            for bh in range(2):
                bsl = slice(bh * 128, (bh + 1) * 128)
                pre = psA.tile([128, GS], F32, tag="ps_scratch", name="pre")
                nc.tensor.matmul(pre[:], xa[0][:, bsl], wi[0][:],
                                 start=True, stop=False)
                nc.tensor.matmul(pre[:], xa[1][:, bsl], wi[1][:],
                                 start=False, stop=False)
                nc.tensor.matmul(pre[:], xa2[:, bsl], wi2[:],
                                 start=False, stop=False)
                nc.tensor.matmul(pre[:], ha[0][:, bsl], wh[0][:],
                                 start=False, stop=False)
                nc.tensor.matmul(pre[:], ha[1][:, bsl], wh[1][:],
                                 start=False, stop=True)
                # sigmoid(x) = 0.5*(1 + tanh(x/2)); tanh is in exp_and_others
                th = sbA.tile([128, 128], F32, tag="th")
                nc.scalar.activation(th[:], pre[:, 0:128], AF.Tanh, scale=0.5)
                gates = sbA.tile([128, GS], F32, tag="gates")
                nc.vector.tensor_scalar(
                    gates[:, 0:128], th[:], 0.5, 0.5,
                    op0=mybir.AluOpType.mult, op1=mybir.AluOpType.add)
                nc.scalar.activation(gates[:, 128:160], pre[:, 128:160],
                                     AF.Tanh)
                # c_part = f*c + i*c~
                fc = sbA.tile([128, HS], F32, tag="fc")
                nc.vector.tensor_mul(fc[:], gates[:, 0:HS], ctile[bh][:])
                ic = sbA.tile([128, HS], F32, tag="ic")
                nc.vector.tensor_mul(ic[:], gates[:, HS:2 * HS],
                                     gates[:, 128:160])
                cp = sbA.tile([128, HS], F32, tag="cp")
                nc.vector.tensor_add(cp[:], fc[:], ic[:])
                nc.sync.dma_start(org.ap()[bsl, 0:HS],
                                  gates[:, 2 * HS:3 * HS])      # o
                nc.sync.dma_start(org.ap()[bsl, HS:2 * HS],
                                  gates[:, 3 * HS:4 * HS])      # r
                nc.sync.dma_start(org.ap()[bsl, 2 * HS:3 * HS], cp[:])

            # --- streamed kNN retrieval ---
            av = [avps.tile([128, H + 2], F32, tag=f"av{bh}", name=f"av{bh}")
                  for bh in range(2)]
            ti = 0
            for off, bs in _blocks():
                nt = bs // 128
                kt = kpool.tile([128, 2, bs], F8, tag="kt", name="kt")
                nc.sync.dma_start(
                    kt[:], k8.ap()[:, 2 * off:2 * off + 2 * bs].rearrange(
                        "p (c l) -> p c l", c=2))
                vt = vpool.tile([128, nt, H + 2], F8, tag="vt", name="vt")
                t0 = off // 128
                nc.sync.dma_start(
                    vt[:], v8.ap()[:, t0 * (H + 2):(t0 + nt) * (H + 2)]
                    .rearrange("p (t h) -> p t h", t=nt))
                for g0 in range(0, nt, GRP):
                    ng = min(GRP, nt - g0)
                    sm = smps.tile([128, GRP, B], F32, tag="sm", name="sm")
                    for j in range(ng):
                        lsl = slice((g0 + j) * 128, (g0 + j + 1) * 128)
                        nc.tensor.matmul(sm[:, j, :], kt[:, 0, lsl], qnT[0][:],
                                         start=True, stop=False)
                        nc.tensor.matmul(sm[:, j, :], kt[:, 1, lsl], qnT[1][:],
                                         start=False, stop=True)
                    ex = expool.tile([128, GRP, B], F8, tag="ex", name="ex")
                    nc.scalar.activation(ex[:, 0:ng, :], sm[:, 0:ng, :],
                                         AF.Exp, bias=cm1[:],
                                         scale=1.0 / (SCALE * SCALE))
                    for j in range(ng):
                        first = ti == 0
                        last = ti == NT_ALL - 1
                        for bh in range(2):
                            nc.tensor.matmul(
                                av[bh][:],
                                ex[:, j, bh * 128:(bh + 1) * 128],
                                vt[:, g0 + j, :],
                                start=first, stop=last)
                        ti += 1

            for bh in range(2):
                avs = sbA.tile([128, H + 2], F32, tag="avs")
                nc.vector.tensor_copy(avs[:], av[bh][:])
                nc.sync.dma_start(nd.ap()[bh * 128:(bh + 1) * 128, :],
                                  avs[:])

    nc.compile()
    return nc


_NC_CACHE = {}


def _get_nc():
    if "nc" not in _NC_CACHE:
        _NC_CACHE["nc"] = _build()
    return _NC_CACHE["nc"]


def _shard_inputs(x_t, h, c, W_i2h, b_i2h, W_h2h, b_h2h, keys, vals):
    f = np.float32
    fp8 = ml_dtypes.float8_e4m3
    bf16 = ml_dtypes.bfloat16
    x_t = np.asarray(x_t, f)
    h = np.asarray(h, f)
    c = np.asarray(c, f)
    W_i2h = np.asarray(W_i2h, f)
    b_i2h = np.asarray(b_i2h, f)
    W_h2h = np.asarray(W_h2h, f)
    b_h2h = np.asarray(b_h2h, f)
    keys = np.asarray(keys, f)
    vals = np.asarray(vals, f)

    # host normalization (matches reference: x / max(||x||, eps))
    qn = x_t / np.maximum(np.linalg.norm(x_t, axis=1, keepdims=True), EPS)
    qnT8 = np.ascontiguousarray(
        (SCALE * qn).T.reshape(2, 128, B).astype(bf16))
    kn = keys / np.maximum(np.linalg.norm(keys, axis=1, keepdims=True), EPS)
    kn8 = (SCALE * kn).astype(fp8)           # [L, D] fp8
    v8f = vals.astype(fp8)                   # [L, H] fp8

    xaT = np.ascontiguousarray(np.concatenate(
        [x_t.T, np.ones((2, B), f)], axis=0)).astype(bf16)
    hTb = np.ascontiguousarray(h.T).astype(bf16)
    WiT_full = W_i2h.T  # [D, G]
    WhT_full = W_h2h.T  # [H, G]

    in_maps = []
    for k in range(NCORES):
        sl = slice(k * L_LOC, (k + 1) * L_LOC)
        kpad = np.zeros((LPAD, D), fp8)
        kpad[:L_LOC] = kn8[sl]
        # k8[p, bi*2*bs + cc*bs + l] = kpad[off+l, cc*128+p]
        parts = []
        for off, bs in _blocks():
            blkT = kpad[off:off + bs].T          # [256, bs]
            parts.append(blkT.reshape(2, 128, bs).transpose(1, 0, 2)
                         .reshape(128, 2 * bs))
        k8a = np.ascontiguousarray(np.concatenate(parts, axis=1))

        vpad = np.zeros((LPAD, H + 2), fp8)
        vpad[:L_LOC, :H] = v8f[sl]
        vpad[:L_LOC, H] = fp8(1.0)  # denominator column; pad rows stay 0
        v8a = np.ascontiguousarray(
            vpad.reshape(NT_ALL, 128, H + 2).transpose(1, 0, 2)
            .reshape(128, NT_ALL * (H + 2)))

        gcols = np.concatenate(
            [np.arange(j * H + k * HS, j * H + (k + 1) * HS)
             for j in range(5)])
        WiTa = np.concatenate(
            [WiT_full[:, gcols], b_i2h[gcols][None, :],
             b_h2h[gcols][None, :]], axis=0).astype(bf16)
        in_maps.append({
            "k8": k8a,
            "v8": v8a,
            "qnT8": qnT8,
            "xaT": xaT,
            "hT": hTb,
            "WiT": np.ascontiguousarray(WiTa),
            "WhT": np.ascontiguousarray(WhT_full[:, gcols].astype(bf16)),
            "c_sl": np.ascontiguousarray(c[:, k * HS:(k + 1) * HS]),
        })
    return in_maps


def kernel(x_t, h, c, W_i2h, b_i2h, W_h2h, b_h2h, keys, vals):
    nc = _get_nc()
    in_maps = _shard_inputs(x_t, h, c, W_i2h, b_i2h, W_h2h, b_h2h, keys, vals)
    res = bass_utils.run_bass_kernel_spmd(
        nc, in_maps, core_ids=list(range(NCORES)))

    num = np.zeros((B, H), np.float64)
    den = np.zeros((B,), np.float64)
    for k in range(NCORES):
        ndk = res.results[k]["nd"]
        num += ndk[:, :H]
        den += ndk[:, H]
    m = np.tanh(num / den[:, None]).astype(np.float32)

    h_t = np.empty((B, H), np.float32)
    c_t = np.empty((B, H), np.float32)
    for k in range(NCORES):
        orgk = res.results[k]["org"]
        o = orgk[:, 0:HS]
        r = orgk[:, HS:2 * HS]
        cp = orgk[:, 2 * HS:3 * HS]
        hs = slice(k * HS, (k + 1) * HS)
        ct = cp + r * m[:, hs]
        c_t[:, hs] = ct
        h_t[:, hs] = o * np.tanh(ct)
    return (h_t, c_t)


# revision 8
# speedup vs baseline: 2.2327x; 1.1855x over previous
"""DND-LSTM cell (retrieval kNN + LSTM gates) on 8 Trainium2 NeuronCores.

Strategy: shard keys/vals along dict_len (L=100000), 12500/core (zero-padded
to 12544). Keys are normalized, scaled by 16 and cast to fp8e4 on the host, so
the device streams 6.5MB/core instead of 25.6MB (memory-bound regime) and
needs no on-device norm chain at all. Queries are normalized+scaled on the
host too (bf16). Flash-softmax with the constant shift "-1" (cosine <= 1):

  ex[l, b]   = exp(sims[l, b] - 1)          sims = (16 kn_l) . (16 qn_b) / 256
  num[b, :]  += ex[l, b] * vals[l, :]        (fp8 vals, f32 PSUM accumulate)
  den[b]     += ex[l, b]                     (via an all-ones vals column)

Per-core device program, per key block (one contiguous fp8 DMA each for
keysT [128, 2, bs] and vals [128, nt, 258]):
  PE   sm[128l, 256b] += kt8[:, c, tile].T @ qnT[c]     (fp8 x bf16, c = 0, 1)
  ACT  ex = exp(sm / 256 - 1) -> fp8, batched 4 l-tiles per ACTIVATE
       (the +352cyc per-instruction overhead amortizes: 287ns/tile vs 507)
  PE   av[b, 0:258] += ex_tile.T @ v8_tile               (persistent PSUM)

The av matmuls are emitted AV_DELAY groups behind their sims group: the PE
queue is strict FIFO, so without the delay the PE idles ~1.3us per group
waiting for the ACT exp it needs before the av matmuls. Block sizes ladder up
(256, 512, 1024, 2048...) so the first sims matmul only waits for a 130KB DMA
instead of 1MB. The zero-padded tail rows contribute exactly 0 (vals rows
incl. den column are 0), so no ragged matmuls are needed.

LSTM gates are sharded over hidden dim (32 cols/core), bf16 weights,
sigmoid/tanh via the native Tanh spline so the whole kernel uses ONE ACT
table (exp_and_others: Exp + Tanh); the gate compute is emitted mid-stream
where its DMAs are long since complete. The host sums the 8 num/den partials
and applies the final elementwise combine.
"""

from collections import deque

import ml_dtypes
import numpy as np

import concourse.bacc as bacc
import concourse.mybir as mybir
import concourse.tile as tile
from concourse import bass_utils

F32 = mybir.dt.float32
BF16 = mybir.dt.bfloat16
F8 = mybir.dt.float8e4
AF = mybir.ActivationFunctionType

B = 256
D = 256
H = 256
NCORES = 8
HS = H // NCORES          # 32 hidden cols per core
GS = 5 * HS               # 160 gate cols per core
L = 100000
L_LOC = L // NCORES       # 12500 real keys per core
LPAD = ((L_LOC + 127) // 128) * 128  # 12544
NT_ALL = LPAD // 128      # 98 l-tiles
GRP = 4                   # l-tiles per batched exp
AVD = 2                   # groups of delay before av consumes ex
SCALE = 16.0              # host scale on kn/qn; exp scale = 1/SCALE^2
EPS = 1e-8

# ladder up so the first matmul waits on a small DMA, then steady 2048
_BLOCKS = [256, 512, 1024] + [2048] * 5 + [512]
assert sum(_BLOCKS) == LPAD

_TABLES_PATCHED = False


def _patch_act_tables():
    """Resolve every ACT function to exp_and_others (has Exp AND Tanh), so
    the kernel performs exactly one 2.7us ACT_TABLE_LOAD."""
    global _TABLES_PATCHED
    if _TABLES_PATCHED:
        return
    _TABLES_PATCHED = True
    orig = bacc.get_activation_tables

    def patched(arch):
        t = dict(orig(arch))
        keep = "exp_and_others"
        if keep in t:
            t = {name: (funcs if name == keep else set())
                 for name, funcs in t.items()}
        return t

    bacc.get_activation_tables = patched


def _blocks():
    out = []
    off = 0
    for bs in _BLOCKS:
        out.append((off, bs))
        off += bs
    return out


def _build():
    _patch_act_tables()
    nc = bacc.Bacc("TRN2", target_bir_lowering=False, debug=False,
                   num_devices=NCORES)

    k8 = nc.dram_tensor("k8", [128, 2 * LPAD], F8, kind="ExternalInput")
    v8 = nc.dram_tensor("v8", [128, NT_ALL * (H + 2)], F8,
                        kind="ExternalInput")
    qnT8 = nc.dram_tensor("qnT8", [2, 128, B], BF16, kind="ExternalInput")
    # hT | WhT | cT packed, partition rows 0:256
    p2 = nc.dram_tensor("p2", [256, B + GS + HS], BF16,
                        kind="ExternalInput")
    # xaT | WiT packed, partition rows 0:258
    p1 = nc.dram_tensor("p1", [D + 2, B + GS], BF16, kind="ExternalInput")

    nd = nc.dram_tensor("nd", [B, H + 2], F32, kind="ExternalOutput")
    org = nc.dram_tensor("org", [B, 3 * HS], F32, kind="ExternalOutput")

    W2 = B + GS + HS       # p2 row width
    W1 = B + GS            # p1 row width

    with tile.TileContext(nc) as tc:
        with (
            tc.tile_pool(name="const", bufs=1) as const,
            tc.tile_pool(name="sbA", bufs=2) as sbA,
            tc.tile_pool(name="psA", bufs=1, space="PSUM") as psA,
            tc.tile_pool(name="kpool", bufs=3) as kpool,
            tc.tile_pool(name="vpool", bufs=3) as vpool,
            tc.tile_pool(name="smps", bufs=2, space="PSUM") as smps,
            tc.tile_pool(name="expool", bufs=4) as expool,
            tc.tile_pool(name="avps", bufs=1, space="PSUM") as avps,
        ):
            cm1 = const.tile([128, 1], F32)
            nc.vector.memset(cm1[:], -1.0)
            # DMA order is issue order on the Sync queue: qnT (needed by the
            # first sims matmul) first, then the first two key/val blocks,
            # then the remaining small inputs.
            qt = const.tile([128, 2, B], BF16, tag="qt", name="qt")
            nc.sync.dma_start(
                qt[:], qnT8.ap().rearrange("c p b -> p c b"))
            qnT = [qt[:, c, :] for c in range(2)]

            blts = _blocks()
            btiles = {}

            def emit_block_dma(bi):
                off, bs = blts[bi]
                nt = bs // 128
                kt = kpool.tile([128, 2, bs], F8, tag="kt", name="kt")
                nc.sync.dma_start(
                    kt[:], k8.ap()[:, 2 * off:2 * off + 2 * bs].rearrange(
                        "p (c l) -> p c l", c=2))
                vt = vpool.tile([128, nt, H + 2], F8, tag="vt", name="vt")
                t0 = off // 128
                nc.sync.dma_start(
                    vt[:], v8.ap()[:, t0 * (H + 2):(t0 + nt) * (H + 2)]
                    .rearrange("p (t h) -> p t h", t=nt))
                btiles[bi] = (kt, vt)

            emit_block_dma(0)
            emit_block_dma(1)

            sm2 = const.tile([128, 2, W2], BF16, tag="sm2", name="sm2")
            nc.sync.dma_start(
                sm2[:], p2.ap().rearrange("(c p) w -> p c w", p=128))
            sm1 = const.tile([128, 2, W1], BF16, tag="sm1", name="sm1")
            nc.sync.dma_start(
                sm1[:], p1.ap()[0:256, :].rearrange("(c p) w -> p c w", p=128))
            sm1b = const.tile([2, W1], BF16, tag="sm1b", name="sm1b")
            nc.sync.dma_start(sm1b[:], p1.ap()[256:258, :])

            ha = [sm2[:, i, 0:B] for i in range(2)]
            wh = [sm2[:, i, B:B + GS] for i in range(2)]
            ctile = [sm2[:, i, B + GS:B + GS + HS] for i in range(2)]
            xa = [sm1[:, i, 0:B] for i in range(2)]
            wi = [sm1[:, i, B:B + GS] for i in range(2)]
            xa2 = sm1b[:, 0:B]
            wi2 = sm1b[:, B:B + GS]

            av = [avps.tile([128, H + 2], F32, tag=f"av{bh}", name=f"av{bh}")
                  for bh in range(2)]

            def emit_lstm():
                # ctile holds bf16(c); gate math in f32
                for bh in range(2):
                    bsl = slice(bh * 128, (bh + 1) * 128)
                    pre = psA.tile([128, GS], F32, tag="ps_scratch",
                                   name="pre")
                    nc.tensor.matmul(pre[:], xa[0][:, bsl], wi[0],
                                     start=True, stop=False)
                    nc.tensor.matmul(pre[:], xa[1][:, bsl], wi[1],
                                     start=False, stop=False)
                    nc.tensor.matmul(pre[:], xa2[:, bsl], wi2,
                                     start=False, stop=False)
                    nc.tensor.matmul(pre[:], ha[0][:, bsl], wh[0],
                                     start=False, stop=False)
                    nc.tensor.matmul(pre[:], ha[1][:, bsl], wh[1],
                                     start=False, stop=True)
                    # sigmoid(x) = 0.5*(1 + tanh(x/2)); tanh is in
                    # exp_and_others so no table switch
                    th = sbA.tile([128, 128], F32, tag="th")
                    nc.scalar.activation(th[:], pre[:, 0:128], AF.Tanh,
                                         scale=0.5)
                    gates = sbA.tile([128, GS], F32, tag="gates")
                    nc.vector.tensor_scalar(
                        gates[:, 0:128], th[:], 0.5, 0.5,
                        op0=mybir.AluOpType.mult, op1=mybir.AluOpType.add)
                    nc.scalar.activation(gates[:, 128:160], pre[:, 128:160],
                                         AF.Tanh)
                    # c_part = f*c + i*c~
                    fc = sbA.tile([128, HS], F32, tag="fc")
                    nc.vector.tensor_mul(fc[:], gates[:, 0:HS], ctile[bh])
                    ic = sbA.tile([128, HS], F32, tag="ic")
                    nc.vector.tensor_mul(ic[:], gates[:, HS:2 * HS],
                                         gates[:, 128:160])
                    cp = sbA.tile([128, HS], F32, tag="cp")
                    nc.vector.tensor_add(cp[:], fc[:], ic[:])
                    nc.sync.dma_start(org.ap()[bsl, 0:2 * HS],
                                      gates[:, 2 * HS:4 * HS])      # o | r
                    nc.sync.dma_start(org.ap()[bsl, 2 * HS:3 * HS], cp[:])

            # --- streamed kNN retrieval, software-pipelined ---
            pend = deque()
            state = {"ti": 0}

            def emit_av(item):
                ex, vt, g0, ng, ti0 = item
                for j in range(ng):
                    first = ti0 + j == 0
                    last = ti0 + j == NT_ALL - 1
                    for bh in range(2):
                        nc.tensor.matmul(
                            av[bh][:],
                            ex[:, j, bh * 128:(bh + 1) * 128],
                            vt[:, g0 + j, :],
                            start=first, stop=last)

            for bi, (off, bs) in enumerate(blts):
                nt = bs // 128
                if bi + 1 < len(blts) and bi + 1 not in btiles:
                    emit_block_dma(bi + 1)
                kt, vt = btiles.pop(bi)
                for g0 in range(0, nt, GRP):
                    ng = min(GRP, nt - g0)
                    sm = smps.tile([128, GRP, B], F32, tag="sm", name="sm")
                    for j in range(ng):
                        lsl = slice((g0 + j) * 128, (g0 + j + 1) * 128)
                        nc.tensor.matmul(sm[:, j, :], kt[:, 0, lsl],
                                         qnT[0], start=True, stop=False)
                        nc.tensor.matmul(sm[:, j, :], kt[:, 1, lsl],
                                         qnT[1], start=False, stop=True)
                    ex = expool.tile([128, GRP, B], F8, tag="ex", name="ex")
                    nc.scalar.activation(ex[:, 0:ng, :], sm[:, 0:ng, :],
                                         AF.Exp, bias=cm1[:],
                                         scale=1.0 / (SCALE * SCALE))
                    pend.append((ex, vt, g0, ng, state["ti"]))
                    state["ti"] += ng
                    if len(pend) > AVD:
                        emit_av(pend.popleft())
                if bi == 2:
                    emit_lstm()
            while pend:
                emit_av(pend.popleft())

            for bh in range(2):
                avs = sbA.tile([128, H + 2], F32, tag="avs")
                nc.vector.tensor_copy(avs[:], av[bh][:])
                nc.sync.dma_start(nd.ap()[bh * 128:(bh + 1) * 128, :],
                                  avs[:])

    nc.compile()
    return nc


_NC_CACHE = {}


def _get_nc():
    if "nc" not in _NC_CACHE:
        _NC_CACHE["nc"] = _build()
    return _NC_CACHE["nc"]


def _shard_inputs(x_t, h, c, W_i2h, b_i2h, W_h2h, b_h2h, keys, vals):
    f = np.float32
    fp8 = ml_dtypes.float8_e4m3
    bf16 = ml_dtypes.bfloat16
    x_t = np.asarray(x_t, f)
    h = np.asarray(h, f)
    c = np.asarray(c, f)
    W_i2h = np.asarray(W_i2h, f)
    b_i2h = np.asarray(b_i2h, f)
    W_h2h = np.asarray(W_h2h, f)
    b_h2h = np.asarray(b_h2h, f)
    keys = np.asarray(keys, f)
    vals = np.asarray(vals, f)

    # host normalization (matches reference: x / max(||x||, eps))
    qn = x_t / np.maximum(np.linalg.norm(x_t, axis=1, keepdims=True), EPS)
    qnT8 = np.ascontiguousarray(
        (SCALE * qn).T.reshape(2, 128, B)).astype(bf16)
    kn = keys / np.maximum(np.linalg.norm(keys, axis=1, keepdims=True), EPS)
    kn8 = (SCALE * kn).astype(fp8)           # [L, D] fp8
    v8f = vals.astype(fp8)                   # [L, H] fp8

    xaT = np.concatenate([x_t.T, np.ones((2, B), f)], axis=0).astype(bf16)
    hTb = h.T.astype(bf16)
    WiT_full = W_i2h.T  # [D, G]
    WhT_full = W_h2h.T  # [H, G]

    in_maps = []
    for k in range(NCORES):
        sl = slice(k * L_LOC, (k + 1) * L_LOC)
        kpad = np.zeros((LPAD, D), fp8)
        kpad[:L_LOC] = kn8[sl]
        parts = []
        for off, bs in _blocks():
            blkT = kpad[off:off + bs].T          # [256, bs]
            parts.append(blkT.reshape(2, 128, bs).transpose(1, 0, 2)
                         .reshape(128, 2 * bs))
        k8a = np.ascontiguousarray(np.concatenate(parts, axis=1))

        vpad = np.zeros((LPAD, H + 2), fp8)
        vpad[:L_LOC, :H] = v8f[sl]
        vpad[:L_LOC, H] = fp8(1.0)  # denominator column; pad rows stay 0
        v8a = np.ascontiguousarray(
            vpad.reshape(NT_ALL, 128, H + 2).transpose(1, 0, 2)
            .reshape(128, NT_ALL * (H + 2)))

        gcols = np.concatenate(
            [np.arange(j * H + k * HS, j * H + (k + 1) * HS)
             for j in range(5)])
        # p2 = hT | WhT | c-as-bf16, rows: c rows are batch index (the DMA
        # just moves rows; c rows 0:128 -> chunk 0, 128:256 -> chunk 1)
        p2 = np.concatenate(
            [hTb, WhT_full[:, gcols].astype(bf16),
             np.ascontiguousarray(
                 c[:, k * HS:(k + 1) * HS]).astype(bf16).reshape(256, HS)],
            axis=1)
        p1 = np.concatenate(
            [xaT,
             np.concatenate([WiT_full[:, gcols], b_i2h[gcols][None, :],
                             b_h2h[gcols][None, :]], axis=0).astype(bf16)],
            axis=1)
        in_maps.append({
            "k8": k8a,
            "v8": v8a,
            "qnT8": qnT8,
            "p2": np.ascontiguousarray(p2),
            "p1": np.ascontiguousarray(p1.astype(bf16)),
        })
    return in_maps


def kernel(x_t, h, c, W_i2h, b_i2h, W_h2h, b_h2h, keys, vals):
    nc = _get_nc()
    in_maps = _shard_inputs(x_t, h, c, W_i2h, b_i2h, W_h2h, b_h2h, keys, vals)
    res = bass_utils.run_bass_kernel_spmd(
        nc, in_maps, core_ids=list(range(NCORES)))

    num = np.zeros((B, H), np.float64)
    den = np.zeros((B,), np.float64)
    for k in range(NCORES):
        ndk = res.results[k]["nd"]
        num += ndk[:, :H]
        den += ndk[:, H]
    m = np.tanh(num / den[:, None]).astype(np.float32)

    h_t = np.empty((B, H), np.float32)
    c_t = np.empty((B, H), np.float32)
    for k in range(NCORES):
        orgk = res.results[k]["org"]
        o = orgk[:, 0:HS]
        r = orgk[:, HS:2 * HS]
        cp = orgk[:, 2 * HS:3 * HS]
        hs = slice(k * HS, (k + 1) * HS)
        ct = cp + r * m[:, hs]
        c_t[:, hs] = ct
        h_t[:, hs] = o * np.tanh(ct)
    return (h_t, c_t)


# revision 11
# speedup vs baseline: 3.3248x; 1.4891x over previous
"""DND-LSTM cell (retrieval kNN + LSTM gates) on 8 Trainium2 NeuronCores.

Strategy: shard keys/vals along dict_len (L=100000), 12500/core (zero-padded
to 12544). Keys are normalized, scaled by 16 and cast to fp8e4 on the host, so
the device streams 6.5MB/core instead of 25.6MB (memory-bound regime) and
needs no on-device norm chain at all. Queries are normalized+scaled+fp8 on the
host too. Flash-softmax with the constant shift "-1" (cosine <= 1):

  ex[l, b]   = exp(sims[l, b] - 1)          sims = (16 kn_l) . (16 qn_b) / 256
  num[b, :]  += ex[l, b] * vals[l, :]        (fp8 vals, f32 PSUM accumulate)
  den[b]     += ex[l, b]                     (via an all-ones vals column)

All heavy matmuls run in fp8 DoubleRow mode (2 MACs/cell/cycle, HW-measured
110ns per sims tile and 158ns per av pair vs 230/222 without):

  PE   sm[128l, 256b] = DR-matmul(kt8[:, :, tile], qn8)     one MM per tile
  ACT  ex = exp(sm / 256 - 1) -> fp8, batched 6 l-tiles per ACTIVATE
       (the ~450ns per-instruction overhead amortizes)
  PE   av[bh][ck] += DR-matmul(ex pair, vt pair chunk)      129-col chunks
       (DoubleRow moving operands are capped at 2x256 free, so the 258 val
       columns split as [h0-127|den] and [h128-255|pad]; vals are laid out
       in that column order on the host)

The av matmuls are emitted AV_DELAY groups behind their sims group: the PE
queue is strict FIFO, so without the delay the PE idles ~1.3us per group
waiting for the ACT exp it needs before the av matmuls. Block sizes ladder up
so the first sims matmul only waits for a small first DMA. The zero-padded
tail rows contribute exactly 0 (vals rows incl. den column are 0), so no
ragged matmuls are needed.

LSTM gates are sharded over hidden dim (32 cols/core), bf16 weights,
sigmoid/tanh via the native Tanh spline so the whole kernel uses ONE ACT
table (exp_and_others: Exp + Tanh); the gate compute is emitted mid-stream
where its DMAs are long since complete. The host sums the 8 num/den partials
and applies the final elementwise combine.
"""

from collections import deque

import ml_dtypes
import numpy as np

import concourse.bacc as bacc
import concourse.mybir as mybir
import concourse.tile as tile
from concourse import bass_utils

F32 = mybir.dt.float32
BF16 = mybir.dt.bfloat16
F8 = mybir.dt.float8e4
AF = mybir.ActivationFunctionType
DR = mybir.MatmulPerfMode.DoubleRow

B = 256
D = 256
H = 256
NCORES = 8
HS = H // NCORES          # 32 hidden cols per core
GS = 5 * HS               # 160 gate cols per core
L = 100000
L_LOC = L // NCORES       # 12500 real keys per core
LPAD = ((L_LOC + 127) // 128) * 128  # 12544
NT_ALL = LPAD // 128      # 98 l-tiles
GRP = 6                   # l-tiles per batched exp (3 PSUM banks)
AVD = 2                   # groups of delay before av consumes ex
SCALE = 16.0              # host scale on kn/qn; exp scale = 1/SCALE^2
EPS = 1e-8
VW = H + 2                # vals row: h0-127 | den | h128-255 | pad

# tile counts per block: ladder up, then multiples of GRP, 2-tile tail
_BT = [6, 12, 24, 24, 24, 6, 2]
assert sum(_BT) == NT_ALL

_TABLES_PATCHED = False


def _patch_act_tables():
    """Resolve every ACT function to exp_and_others (has Exp AND Tanh), so
    the kernel performs exactly one ACT_TABLE_LOAD."""
    global _TABLES_PATCHED
    if _TABLES_PATCHED:
        return
    _TABLES_PATCHED = True
    orig = bacc.get_activation_tables

    def patched(arch):
        t = dict(orig(arch))
        keep = "exp_and_others"
        if keep in t:
            t = {name: (funcs if name == keep else set())
                 for name, funcs in t.items()}
        return t

    bacc.get_activation_tables = patched


def _blocks():
    out = []
    off = 0
    for nt in _BT:
        out.append((off, nt * 128))
        off += nt * 128
    return out


def _build():
    _patch_act_tables()
    nc = bacc.Bacc("TRN2", target_bir_lowering=False, debug=False,
                   num_devices=NCORES)

    k8 = nc.dram_tensor("k8", [128, 2 * LPAD], F8, kind="ExternalInput")
    v8 = nc.dram_tensor("v8", [128, NT_ALL * VW], F8, kind="ExternalInput")
    qn8 = nc.dram_tensor("qn8", [128, 2, B], F8, kind="ExternalInput")
    # hT | WhT | cT packed, partition rows 0:256
    p2 = nc.dram_tensor("p2", [256, B + GS + HS], BF16,
                        kind="ExternalInput")
    # xaT | WiT packed, partition rows 0:258
    p1 = nc.dram_tensor("p1", [D + 2, B + GS], BF16, kind="ExternalInput")

    nd = nc.dram_tensor("nd", [B, VW], F32, kind="ExternalOutput")
    org = nc.dram_tensor("org", [B, 3 * HS], F32, kind="ExternalOutput")

    W2 = B + GS + HS       # p2 row width
    W1 = B + GS            # p1 row width

    with tile.TileContext(nc) as tc:
        with (
            tc.tile_pool(name="const", bufs=1) as const,
            tc.tile_pool(name="sbA", bufs=2) as sbA,
            tc.tile_pool(name="kpool", bufs=3) as kpool,
            tc.tile_pool(name="vpool", bufs=3) as vpool,
            tc.tile_pool(name="smps", bufs=2, space="PSUM") as smps,
            tc.tile_pool(name="expool", bufs=4) as expool,
            tc.tile_pool(name="avps", bufs=1, space="PSUM") as avps,
        ):
            cm1 = const.tile([128, 1], F32)
            nc.vector.memset(cm1[:], -1.0)
            # DMA order is issue order on the Sync queue: qn8 (needed by the
            # first sims matmul) first, then the first two key/val blocks,
            # then the remaining small inputs.
            qt = const.tile([128, 2, B], F8, tag="qt", name="qt")
            nc.sync.dma_start(qt[:], qn8.ap())

            blts = _blocks()
            btiles = {}

            def emit_block_dma(bi):
                off, bs = blts[bi]
                nt = bs // 128
                kt = kpool.tile([128, 2, bs], F8, tag="kt", name="kt")
                nc.sync.dma_start(
                    kt[:], k8.ap()[:, 2 * off:2 * off + 2 * bs].rearrange(
                        "p (c l) -> p c l", c=2))
                vt = vpool.tile([128, nt, VW], F8, tag="vt", name="vt")
                t0 = off // 128
                nc.sync.dma_start(
                    vt[:], v8.ap()[:, t0 * VW:(t0 + nt) * VW]
                    .rearrange("p (t h) -> p t h", t=nt))
                btiles[bi] = (kt, vt)

            emit_block_dma(0)
            emit_block_dma(1)

            sm2 = const.tile([128, 2, W2], BF16, tag="sm2", name="sm2")
            nc.sync.dma_start(
                sm2[:], p2.ap().rearrange("(c p) w -> p c w", p=128))
            sm1 = const.tile([128, 2, W1], BF16, tag="sm1", name="sm1")
            nc.sync.dma_start(
                sm1[:], p1.ap()[0:256, :].rearrange("(c p) w -> p c w", p=128))
            sm1b = const.tile([2, W1], BF16, tag="sm1b", name="sm1b")
            nc.sync.dma_start(sm1b[:], p1.ap()[256:258, :])

            ha = [sm2[:, i, 0:B] for i in range(2)]
            wh = [sm2[:, i, B:B + GS] for i in range(2)]
            ctile = [sm2[:, i, B + GS:B + GS + HS] for i in range(2)]
            xa = [sm1[:, i, 0:B] for i in range(2)]
            wi = [sm1[:, i, B:B + GS] for i in range(2)]
            xa2 = sm1b[:, 0:B]
            wi2 = sm1b[:, B:B + GS]

            # av accumulators: two 129-col chunks per bh. PSUM pool tiles are
            # bank-aligned, so bh0's bank-sized tile also hosts the LSTM
            # `pre` scratch in its spare columns (disjoint byte ranges).
            avA = avps.tile([128, 512], F32, tag="avA", name="avA")
            avB = avps.tile([128, 2, 129], F32, tag="avB", name="avB")
            avsl = [[avA[:, 0:129], avA[:, 129:258]],
                    [avB[:, 0, :], avB[:, 1, :]]]
            pre_t = avA[:, 258:258 + GS]

            def emit_lstm():
                for bh in range(2):
                    bsl = slice(bh * 128, (bh + 1) * 128)
                    pre = pre_t
                    nc.tensor.matmul(pre[:], xa[0][:, bsl], wi[0],
                                     start=True, stop=False)
                    nc.tensor.matmul(pre[:], xa[1][:, bsl], wi[1],
                                     start=False, stop=False)
                    nc.tensor.matmul(pre[:], xa2[:, bsl], wi2,
                                     start=False, stop=False)
                    nc.tensor.matmul(pre[:], ha[0][:, bsl], wh[0],
                                     start=False, stop=False)
                    nc.tensor.matmul(pre[:], ha[1][:, bsl], wh[1],
                                     start=False, stop=True)
                    # sigmoid(x) = 0.5*(1 + tanh(x/2)); tanh is in
                    # exp_and_others so no table switch
                    th = sbA.tile([128, 128], F32, tag="th")
                    nc.scalar.activation(th[:], pre[:, 0:128], AF.Tanh,
                                         scale=0.5)
                    gates = sbA.tile([128, GS], F32, tag="gates")
                    nc.vector.tensor_scalar(
                        gates[:, 0:128], th[:], 0.5, 0.5,
                        op0=mybir.AluOpType.mult, op1=mybir.AluOpType.add)
                    nc.scalar.activation(gates[:, 128:160], pre[:, 128:160],
                                         AF.Tanh)
                    # c_part = f*c + i*c~
                    fc = sbA.tile([128, HS], F32, tag="fc")
                    nc.vector.tensor_mul(fc[:], gates[:, 0:HS], ctile[bh])
                    ic = sbA.tile([128, HS], F32, tag="ic")
                    nc.vector.tensor_mul(ic[:], gates[:, HS:2 * HS],
                                         gates[:, 128:160])
                    cp = sbA.tile([128, HS], F32, tag="cp")
                    nc.vector.tensor_add(cp[:], fc[:], ic[:])
                    nc.sync.dma_start(org.ap()[bsl, 0:2 * HS],
                                      gates[:, 2 * HS:4 * HS])      # o | r
                    nc.sync.dma_start(org.ap()[bsl, 2 * HS:3 * HS], cp[:])

            # --- streamed kNN retrieval, DR matmuls, software-pipelined ---
            pend = deque()
            state = {"pair": 0}
            npair = NT_ALL // 2

            def emit_av(item):
                ex, vt, g0, ng, p0 = item
                for p in range(ng // 2):
                    first = p0 + p == 0
                    last = p0 + p == npair - 1
                    for bh in range(2):
                        for ck in range(2):
                            nc.tensor.matmul(
                                avsl[bh][ck],
                                ex[:, 2 * p:2 * p + 2,
                                   bh * 128:(bh + 1) * 128],
                                vt[:, g0 + 2 * p:g0 + 2 * p + 2,
                                   ck * 129:(ck + 1) * 129],
                                start=first, stop=last, perf_mode=DR)

            for bi, (off, bs) in enumerate(blts):
                nt = bs // 128
                if bi + 1 < len(blts) and bi + 1 not in btiles:
                    emit_block_dma(bi + 1)
                kt, vt = btiles.pop(bi)
                for g0 in range(0, nt, GRP):
                    ng = min(GRP, nt - g0)
                    sm = smps.tile([128, GRP, B], F32, tag="sm", name="sm")
                    for j in range(ng):
                        lsl = slice((g0 + j) * 128, (g0 + j + 1) * 128)
                        nc.tensor.matmul(sm[:, j, :], kt[:, :, lsl], qt[:],
                                         start=True, stop=True, perf_mode=DR)
                    ex = expool.tile([128, GRP, B], F8, tag="ex", name="ex")
                    nc.scalar.activation(ex[:, 0:ng, :], sm[:, 0:ng, :],
                                         AF.Exp, bias=cm1[:],
                                         scale=1.0 / (SCALE * SCALE))
                    pend.append((ex, vt, g0, ng, state["pair"]))
                    state["pair"] += ng // 2
                    if len(pend) > AVD:
                        emit_av(pend.popleft())
                if bi == 2:
                    emit_lstm()
            while pend:
                emit_av(pend.popleft())

            for bh in range(2):
                avs = sbA.tile([128, 2 * 129], F32, tag="avs")
                nc.vector.tensor_copy(avs[:, 0:129], avsl[bh][0])
                nc.vector.tensor_copy(avs[:, 129:258], avsl[bh][1])
                nc.sync.dma_start(nd.ap()[bh * 128:(bh + 1) * 128, :],
                                  avs[:])

    nc.compile()
    return nc


_NC_CACHE = {}


def _get_nc():
    if "nc" not in _NC_CACHE:
        _NC_CACHE["nc"] = _build()
    return _NC_CACHE["nc"]


def _shard_inputs(x_t, h, c, W_i2h, b_i2h, W_h2h, b_h2h, keys, vals):
    f = np.float32
    fp8 = ml_dtypes.float8_e4m3
    bf16 = ml_dtypes.bfloat16
    x_t = np.asarray(x_t, f)
    h = np.asarray(h, f)
    c = np.asarray(c, f)
    W_i2h = np.asarray(W_i2h, f)
    b_i2h = np.asarray(b_i2h, f)
    W_h2h = np.asarray(W_h2h, f)
    b_h2h = np.asarray(b_h2h, f)
    keys = np.asarray(keys, f)
    vals = np.asarray(vals, f)

    # host normalization (matches reference: x / max(||x||, eps))
    qn = x_t / np.maximum(np.linalg.norm(x_t, axis=1, keepdims=True), EPS)
    qn8 = np.ascontiguousarray(
        (SCALE * qn).T.reshape(2, 128, B).transpose(1, 0, 2)).astype(fp8)
    kn = keys / np.maximum(np.linalg.norm(keys, axis=1, keepdims=True), EPS)
    kn8 = (SCALE * kn).astype(fp8)           # [L, D] fp8
    v8f = vals.astype(fp8)                   # [L, H] fp8

    xaT = np.concatenate([x_t.T, np.ones((2, B), f)], axis=0).astype(bf16)
    hTb = h.T.astype(bf16)
    WiT_full = W_i2h.T  # [D, G]
    WhT_full = W_h2h.T  # [H, G]

    in_maps = []
    for k in range(NCORES):
        sl = slice(k * L_LOC, (k + 1) * L_LOC)
        kpad = np.zeros((LPAD, D), fp8)
        kpad[:L_LOC] = kn8[sl]
        parts = []
        for off, bs in _blocks():
            blkT = kpad[off:off + bs].T          # [256, bs]
            parts.append(blkT.reshape(2, 128, bs).transpose(1, 0, 2)
                         .reshape(128, 2 * bs))
        k8a = np.ascontiguousarray(np.concatenate(parts, axis=1))

        # vals row layout: h0-127 | den | h128-255 | pad (129-col DR chunks)
        vpad = np.zeros((LPAD, VW), fp8)
        vpad[:L_LOC, 0:128] = v8f[sl][:, 0:128]
        vpad[:L_LOC, 128] = fp8(1.0)  # denominator; pad rows stay 0
        vpad[:L_LOC, 129:257] = v8f[sl][:, 128:256]
        v8a = np.ascontiguousarray(
            vpad.reshape(NT_ALL, 128, VW).transpose(1, 0, 2)
            .reshape(128, NT_ALL * VW))

        gcols = np.concatenate(
            [np.arange(j * H + k * HS, j * H + (k + 1) * HS)
             for j in range(5)])
        # p2 = hT | WhT | c-as-bf16 (c rows are batch index; the DMA just
        # moves rows: c rows 0:128 -> chunk 0, 128:256 -> chunk 1)
        p2 = np.concatenate(
            [hTb, WhT_full[:, gcols].astype(bf16),
             np.ascontiguousarray(
                 c[:, k * HS:(k + 1) * HS]).astype(bf16).reshape(256, HS)],
            axis=1)
        p1 = np.concatenate(
            [xaT,
             np.concatenate([WiT_full[:, gcols], b_i2h[gcols][None, :],
                             b_h2h[gcols][None, :]], axis=0).astype(bf16)],
            axis=1)
        in_maps.append({
            "k8": k8a,
            "v8": v8a,
            "qn8": qn8,
            "p2": np.ascontiguousarray(p2),
            "p1": np.ascontiguousarray(p1.astype(bf16)),
        })
    return in_maps


def kernel(x_t, h, c, W_i2h, b_i2h, W_h2h, b_h2h, keys, vals):
    nc = _get_nc()
    in_maps = _shard_inputs(x_t, h, c, W_i2h, b_i2h, W_h2h, b_h2h, keys, vals)
    res = bass_utils.run_bass_kernel_spmd(
        nc, in_maps, core_ids=list(range(NCORES)))

    num = np.zeros((B, H), np.float64)
    den = np.zeros((B,), np.float64)
    for k in range(NCORES):
        ndk = res.results[k]["nd"]
        num[:, 0:128] += ndk[:, 0:128]
        num[:, 128:256] += ndk[:, 129:257]
        den += ndk[:, 128]
    m = np.tanh(num / den[:, None]).astype(np.float32)

    h_t = np.empty((B, H), np.float32)
    c_t = np.empty((B, H), np.float32)
    for k in range(NCORES):
        orgk = res.results[k]["org"]
        o = orgk[:, 0:HS]
        r = orgk[:, HS:2 * HS]
        cp = orgk[:, 2 * HS:3 * HS]
        hs = slice(k * HS, (k + 1) * HS)
        ct = cp + r * m[:, hs]
        c_t[:, hs] = ct
        h_t[:, hs] = o * np.tanh(ct)
    return (h_t, c_t)
